# revision 43
# baseline (speedup 1.0000x reference)
"""Causal multi-head attention (B=4, S=2048, D=1024, H=16) on 8 NeuronCores.

Sharding: core c = (batch b = c//2, head-group hg = c%2). Each core computes
8 heads of one batch: QKV projection (bf16 matmuls), causal flash-style
attention (bf16 matmuls, exp-without-max softmax with a ones-column
denominator), and a row-parallel out-projection partial. Host sums the two
head-group partials per batch and transposes.

Layouts are feature-major ([feature, token]) except v (token-major) so
attn@v needs no transposes. Head pairs are packed into PE row groups
(rows 0-63 / 64-127) for the K=64 matmuls; PSUM tiles are 2 banks wide
(even head in columns 0-511, odd in 512-1023) so one ACT exp covers both
heads.

Schedule: qi-outer attention rounds (round qi = 4 head-pair blocks); each
pair's q/k token-quarter projections are emitted just before its block
(q quarters map 1:1 to rounds), v groups just before the round that needs
them, and ready out-projection groups drain as fillers inside later blocks
so the PE never idles while ACT chews exps. Attention blocks are
software-pipelined three steps deep (ao matmuls for step kt emitted after
the exp of step kt+3; scores rotate through 3 PSUM buffers, ao through 1).
DMA issue is spread across the sync/gpsimd/scalar queues so descriptor
generation parallelizes, critical transfers before bulk. The
out-projection is one K=128 matmul per head pair accumulating in PSUM;
its epilogue is a single DVE bias-add, keeping ACT exclusively on exps.
"""
import numpy as np
from contextlib import ExitStack

import ml_dtypes

B, S, D, H = 4, 2048, 1024, 16
HD = 64            # head dim
HPC = 8            # heads per core
F = HPC * HD       # 512 features per head-group
QT = 512           # q tile (free dim)
NQI = S // QT      # 4
NKT = S // 128     # 16
NDK = D // 128     # 8 contraction tiles for projections
SCALE = HD ** -0.5

_CACHE = {}


def _build():
    import concourse.bacc as bacc
    import concourse.tile as tile
    import concourse.mybir as mybir

    f32 = mybir.dt.float32
    bf16 = mybir.dt.bfloat16
    EXP = mybir.ActivationFunctionType.Exp

    nc = bacc.Bacc("TRN2", target_bir_lowering=False, debug=False)
    xT = nc.dram_tensor("xT", [D, S], bf16, kind="ExternalInput").ap()
    w_sl = nc.dram_tensor("w_sl", [D, 3 * F], bf16, kind="ExternalInput").ap()
    wo_sl = nc.dram_tensor("wo_sl", [F, D], bf16, kind="ExternalInput").ap()
    bias_t = nc.dram_tensor("bias_t", [128, 8], f32, kind="ExternalInput").ap()
    mask2 = nc.dram_tensor("mask2", [128, 256], bf16, kind="ExternalInput").ap()
    out = nc.dram_tensor("out", [D, S], bf16, kind="ExternalOutput").ap()

    with tile.TileContext(nc) as tc:
        with ExitStack() as ctx:
            misc = ctx.enter_context(tc.tile_pool(name="misc", bufs=1))
            mask_sb = misc.tile([128, 256], bf16, name="mask_sb", tag="mask")
            bias_sb = misc.tile([128, 8], f32, name="bias_sb", tag="bias")

            pwo = ctx.enter_context(tc.tile_pool(name="pwo", bufs=1))
            wo_t = [pwo.tile([128, D], bf16, name=f"wo{g}", tag=f"wo{g}")
                    for g in range(4)]

            # ---- x / w tiles. DMA issue is spread across engine queues so
            # descriptor generation parallelizes (the Sync sequencer alone
            # serializes at ~600ns/transfer): sync feeds the fast-start x
            # quarter 0, gpsimd feeds the merged q|k|v weight tiles + wo,
            # scalar feeds the bulk x quarters 1-3 (3KB lines).
            xw = ctx.enter_context(tc.tile_pool(name="xw", bufs=1))
            x_t = [xw.tile([128, S], bf16, name=f"x{kk}", tag=f"x{kk}")
                   for kk in range(NDK)]
            w_t = [xw.tile([128, 3 * F], bf16, name=f"w{kk}", tag=f"w{kk}")
                   for kk in range(NDK)]
            # A transfer's latency is its line count on one ring, so the
            # round-0-critical pieces are split across rows (parallel rings)
            # and issued from four engine queues concurrently.
            # Critical stream first, in parallel across the three DMA-capable
            # queues; bulk (x quarters 1-3, wo) strictly after so it queues
            # behind the critical pieces in the per-ring FIFOs.
            for kk in range(NDK):
                r0 = slice(kk * 128, (kk + 1) * 128)
                nc.sync.dma_start(x_t[kk][:, 0:QT], xT[r0, 0:QT])
                eng = nc.gpsimd if kk < 4 else nc.scalar
                for h in range(2):
                    rw = slice(kk * 128 + h * 64, kk * 128 + (h + 1) * 64)
                    eng.dma_start(w_t[kk][h * 64:(h + 1) * 64, 0:2 * F],
                                  w_sl[rw, 0:2 * F])
            for kk in range(NDK):
                r0 = slice(kk * 128, (kk + 1) * 128)
                nc.sync.dma_start(w_t[kk][:, 2 * F:3 * F],
                                  w_sl[r0, 2 * F:3 * F])
            nc.gpsimd.dma_start(mask_sb[:], mask2)
            nc.gpsimd.dma_start(bias_sb[:], bias_t)
            for kk in range(NDK):
                r0 = slice(kk * 128, (kk + 1) * 128)
                nc.sync.dma_start(x_t[kk][:, QT:2 * QT], xT[r0, QT:2 * QT])
            for kk in range(NDK):
                r0 = slice(kk * 128, (kk + 1) * 128)
                nc.sync.dma_start(x_t[kk][:, 2 * QT:S], xT[r0, 2 * QT:S])
            # wo rides the end of the sync issue queue (~19us in) so its
            # long transfers never contend with the wv/x critical rings;
            # first use (out-proj fillers) is ~100us in
            for g in range(4):
                nc.sync.dma_start(wo_t[g][:], wo_sl[g * 128:(g + 1) * 128, :])

            pqk = ctx.enter_context(tc.tile_pool(name="pqk", bufs=1))
            pv = ctx.enter_context(tc.tile_pool(name="pv", bufs=1))
            patt = ctx.enter_context(tc.tile_pool(name="patt", bufs=1))
            pP = ctx.enter_context(tc.tile_pool(name="pP", bufs=5))
            pr = ctx.enter_context(tc.tile_pool(name="pr", bufs=2))
            prr = ctx.enter_context(tc.tile_pool(name="prr", bufs=2))
            pstg = ctx.enter_context(tc.tile_pool(name="pstg", bufs=3))

            q_sb = [pqk.tile([128, S], bf16, name=f"q{g}", tag=f"q{g}")
                    for g in range(4)]
            k_sb = [pqk.tile([128, S], bf16, name=f"k{g}", tag=f"k{g}")
                    for g in range(4)]
            v_sb = [pv.tile([128, HPC * (HD + 1)], bf16, name=f"v{t}",
                            tag=f"v{t}") for t in range(NKT)]
            am_t = {}
            for g in range(4):
                for qi in range(NQI):
                    am_t[(g, qi)] = patt.tile(
                        [128, QT], bf16, name=f"am{g}{qi}", tag=f"am{g}{qi}")

            # PSUM: tag "big" = scores + qk/v/out-proj tiles (3 bufs, 6
            # banks); tag "ao" = attention accumulators (1 buf, 2 banks).
            psum = ctx.enter_context(
                tc.tile_pool(name="psum", bufs=2, space="PSUM"))

            def v_group(t2):
                ps = psum.tile([128, 2 * QT], f32, name=f"pv{t2}", tag="big", bufs=3)
                for kk in range(NDK):
                    for j in range(2):
                        tt = 2 * t2 + j
                        nc.tensor.matmul(
                            ps[:, j * F:j * F + F],
                            x_t[kk][:, tt * 128:(tt + 1) * 128],
                            w_t[kk][:, 2 * F:3 * F],
                            start=(kk == 0), stop=(kk == NDK - 1))
                for j in range(2):
                    tt = 2 * t2 + j
                    vv = v_sb[tt].rearrange("p (h c) -> p h c", h=HPC)
                    pp = ps[:, j * F:j * F + F].rearrange(
                        "p (h c) -> p h c", h=HPC)
                    nc.vector.tensor_copy(vv[:, :, 0:HD], pp[:])
                    nc.vector.memset(vv[:, :, HD:HD + 1], 1.0)

            def qk_q(g, part, tg):
                """One token quarter of q (part=0) or k (part=1) for pair g."""
                dest = q_sb if part == 0 else k_sb
                fcol = part * F + g * 128
                ps = psum.tile([128, QT], f32,
                               name=f"pq{part}{g}{tg}", tag="big", bufs=3)
                for kk in range(NDK):
                    nc.tensor.matmul(
                        ps[:], w_t[kk][:, fcol:fcol + 128],
                        x_t[kk][:, tg * QT:(tg + 1) * QT],
                        start=(kk == 0), stop=(kk == NDK - 1))
                nc.vector.tensor_copy(
                    dest[g][:, tg * QT:(tg + 1) * QT], ps[:])

            def outproj(dt, qi):
                """One D-row block (128) for one q tile: am rows 0:128 are
                the head pair's full feature set, so each pair is a single
                K=128 matmul; epilogue = one DVE bias-add from PSUM."""
                dcol = slice(dt * 128, dt * 128 + 128)
                ps = psum.tile([128, QT], f32, name=f"op{dt}{qi}", tag="big", bufs=3)
                for pg in range(4):
                    nc.tensor.matmul(
                        ps[:], wo_t[pg][:, dcol], am_t[(pg, qi)][:],
                        start=(pg == 0), stop=(pg == 3))
                s2 = pstg.tile([128, QT], bf16, name=f"s2{dt}{qi}", tag="s2")
                nc.vector.tensor_scalar_add(s2[:], ps[:],
                                            bias_sb[:, dt:dt + 1])
                nc.sync.dma_start(
                    out[dt * 128:(dt + 1) * 128,
                        qi * QT:(qi + 1) * QT], s2[:])

            fillers = []

            def drain(n):
                for _ in range(n):
                    if fillers:
                        fillers.pop(0)()

            def attn_block(pg, qi):
                """Scores + exp + attn@v + normalize for head pair pg,
                q-range [qi*QT, (qi+1)*QT). Software-pipelined: ao matmuls
                for step kt are emitted after exp of step kt+1; one filler
                unit drains every other step."""
                nkt = 4 * qi + 4
                qs = qi * QT
                he, ho = 2 * pg, 2 * pg + 1
                C = HD + 1
                ao = psum.tile([HD + 1, 2 * QT], f32,
                               name=f"ao{pg}{qi}", tag="ao", bufs=1)
                pend = []

                def emit_ao(pt, kt, n0):
                    st = (kt == 0)
                    sp = (kt == nkt - 1)
                    nc.tensor.matmul(
                        ao[:, n0:QT], v_sb[kt][:, he * C:(he + 1) * C],
                        pt[:, n0:QT], start=st, stop=sp)
                    nc.tensor.matmul(
                        ao[:, QT + n0:2 * QT],
                        v_sb[kt][:, ho * C:(ho + 1) * C],
                        pt[:, QT + n0:2 * QT], start=st, stop=sp)

                for kt in range(nkt):
                    d = kt - 4 * qi
                    n0 = 0 if d < 0 else 128 * d
                    kcol = slice(kt * 128, kt * 128 + 128)
                    sc = psum.tile([128, 2 * QT], f32,
                                   name=f"sc{pg}{qi}{kt}", tag="big", bufs=3)
                    nc.tensor.matmul(
                        sc[:, n0:QT], k_sb[pg][0:64, kcol],
                        q_sb[pg][0:64, qs + n0:qs + QT],
                        start=True, stop=True)
                    nc.tensor.matmul(
                        sc[:, QT + n0:2 * QT], k_sb[pg][64:128, kcol],
                        q_sb[pg][64:128, qs + n0:qs + QT],
                        start=True, stop=True)
                    pt = pP.tile([128, 2 * QT], bf16,
                                 name=f"pt{pg}{qi}{kt}", tag="P")
                    sc3 = sc.rearrange("p (h c) -> p h c", h=2)
                    pt3 = pt.rearrange("p (h c) -> p h c", h=2)
                    nc.scalar.activation(pt3[:, :, n0:QT], sc3[:, :, n0:QT],
                                         EXP, scale=SCALE)
                    if d >= 0:
                        m3 = mask_sb.rearrange("p (h c) -> p h c", h=2)
                        nc.vector.tensor_mul(pt3[:, :, n0:n0 + 128],
                                             pt3[:, :, n0:n0 + 128], m3[:])
                    if len(pend) == 3:
                        emit_ao(*pend.pop(0))
                    if kt % 4 == 1:
                        drain(1)
                    pend.append((pt, kt, n0))
                for p in pend:
                    emit_ao(*p)

                # normalize: 1/rowsum (row HD) via fast recip + gpsimd bcast,
                # split into halves so the two broadcasts pipeline with the
                # copies/recips instead of serializing the whole chain
                am = am_t[(pg, qi)]
                srow = prr.tile([1, 2 * QT], f32, name=f"sr{pg}{qi}", tag="sr")
                rb = pr.tile([HD, 2 * QT], f32, name=f"rb{pg}{qi}", tag="r")
                for h in range(2):
                    cs = slice(h * QT, (h + 1) * QT)
                    nc.vector.tensor_copy(srow[:, cs], ao[HD:HD + 1, cs])
                    nc.vector.reciprocal_approx_fast(srow[:, cs],
                                                     srow[:, cs])
                    nc.gpsimd.partition_broadcast(rb[:, cs], srow[:, cs],
                                                  channels=HD)
                nc.vector.tensor_mul(am[0:64, :], ao[0:HD, 0:QT], rb[:, 0:QT])
                nc.vector.tensor_mul(am[64:128, :], ao[0:HD, QT:2 * QT],
                                     rb[:, QT:2 * QT])

            # ---- schedule ----
            # Round qi: per pair, emit that pair's q/k token-quarter qi just
            # before its block (q quarters map 1:1 to rounds; k quarter qi
            # first needed by the round-qi diagonal). v groups and ready
            # out-proj groups drain as fillers inside the blocks.
            qk_q(0, 0, 0)
            qk_q(0, 1, 0)
            v_group(0)
            v_group(1)
            for qi in range(NQI):
                if qi >= 1:
                    v_group(2 * qi)
                    v_group(2 * qi + 1)
                    # out-proj groups drain one round late (round qi drains
                    # op(qi-2); round 3 also op(2)) so the filler supply
                    # reaches each round's ACT-bound later blocks
                    if qi >= 2:
                        fillers.extend([
                            (lambda dt=dt, q=qi - 2: outproj(dt, q))
                            for dt in range(8)])
                    if qi == 3:
                        fillers.extend([
                            (lambda dt=dt: outproj(dt, 2))
                            for dt in range(8)])
                    qk_q(0, 0, qi)
                    qk_q(0, 1, qi)
                # one-block lookahead: the next pair's q/k units emit before
                # this block so their DVE copies finish under this block's
                # attention instead of stalling the next block's scores
                for g in range(4):
                    if g < 3:
                        qk_q(g + 1, 0, qi)
                        qk_q(g + 1, 1, qi)
                    attn_block(g, qi)
            drain(99)

            # tail: qi=3 out-proj as dt-pairs on 2-bank tiles. The pg0-2
            # accumulation opens before the last block's normalize lands
            # (on both free PSUM tags so neither waits it); pg3 + epilogue
            # close after.
            def op3_open(d0, tag):
                ps = psum.tile([128, 2 * QT], f32, name=f"ot{d0}", tag=tag, bufs=3)
                for j in range(2):
                    dcol = slice((d0 + j) * 128, (d0 + j + 1) * 128)
                    for pg in range(3):
                        nc.tensor.matmul(
                            ps[:, j * QT:(j + 1) * QT], wo_t[pg][:, dcol],
                            am_t[(pg, 3)][:], start=(pg == 0), stop=False)
                return ps

            def op3_close(d0, ps):
                for j in range(2):
                    dcol = slice((d0 + j) * 128, (d0 + j + 1) * 128)
                    nc.tensor.matmul(
                        ps[:, j * QT:(j + 1) * QT], wo_t[3][:, dcol],
                        am_t[(3, 3)][:], start=False, stop=True)
                for j in range(2):
                    dt = d0 + j
                    s2 = pstg.tile([128, QT], bf16, name=f"s3{dt}", tag="s2")
                    nc.vector.tensor_scalar_add(
                        s2[:], ps[:, j * QT:(j + 1) * QT],
                        bias_sb[:, dt:dt + 1])
                    nc.sync.dma_start(
                        out[dt * 128:(dt + 1) * 128, 3 * QT:4 * QT], s2[:])

            ps0 = op3_open(0, "big")
            ps1 = op3_open(2, "big")
            op3_close(0, ps0)
            op3_close(2, ps1)
            ps2 = op3_open(4, "big")
            ps3 = op3_open(6, "big")
            op3_close(4, ps2)
            op3_close(6, ps3)

    nc.compile()
    return nc


def _get_nc():
    if "nc" not in _CACHE:
        _CACHE["nc"] = _build()
    return _CACHE["nc"]


def _prep_inputs(x, w_qkv, w_out, b_out):
    """Build the 8 per-core input maps."""
    x = np.asarray(x, dtype=np.float32)
    w_qkv = np.asarray(w_qkv, dtype=np.float32)
    w_out = np.asarray(w_out, dtype=np.float32)
    b_out = np.asarray(b_out, dtype=np.float32)

    bf = ml_dtypes.bfloat16
    tri = np.triu(np.ones((128, 128), dtype=np.float32))
    mask2 = np.tile(tri, (1, 2)).astype(bf)
    zeros_bias = np.zeros((128, 8), dtype=np.float32)
    bias_t = np.ascontiguousarray(b_out.reshape(8, 128).T)

    in_maps = []
    for c in range(8):
        b, hg = c // 2, c % 2
        cols = hg * F
        w_cat = np.concatenate([
            w_qkv[:, cols:cols + F],
            w_qkv[:, D + cols:D + cols + F],
            w_qkv[:, 2 * D + cols:2 * D + cols + F],
        ], axis=1)
        in_maps.append({
            "xT": np.ascontiguousarray(x[b].T).astype(bf),
            "w_sl": np.ascontiguousarray(w_cat).astype(bf),
            "wo_sl": np.ascontiguousarray(w_out[cols:cols + F, :]).astype(bf),
            "bias_t": bias_t if hg == 0 else zeros_bias,
            "mask2": mask2,
        })
    return in_maps


def _run(inputs, trace=False):
    from concourse.bass_utils import run_bass_kernel_spmd

    nc = _get_nc()
    in_maps = _prep_inputs(**inputs)
    res = run_bass_kernel_spmd(nc, in_maps, core_ids=list(range(8)),
                               trace=trace)
    outs = []
    for b in range(B):
        o = (res.results[2 * b]["out"].astype(np.float32)
             + res.results[2 * b + 1]["out"].astype(np.float32))
        outs.append(o.T)
    full = np.stack(outs).astype(np.float32)
    return full, res


def kernel(x, w_qkv, w_out, b_out):
    full, _ = _run({"x": x, "w_qkv": w_qkv, "w_out": w_out, "b_out": b_out})
    return full


# revision 44
# speedup vs baseline: 1.0058x; 1.0058x over previous
"""Causal multi-head attention (B=4, S=2048, D=1024, H=16) on 8 NeuronCores.

Sharding: core c = (batch b = c//2, head-group hg = c%2). Each core computes
8 heads of one batch: QKV projection (bf16 matmuls), causal flash-style
attention (bf16 matmuls, exp-without-max softmax with a ones-column
denominator), and a row-parallel out-projection partial. Host sums the two
head-group partials per batch and transposes.

Layouts are feature-major ([feature, token]) except v (token-major) so
attn@v needs no transposes. Head pairs are packed into PE row groups
(rows 0-63 / 64-127) for the K=64 matmuls; PSUM tiles are 2 banks wide
(even head in columns 0-511, odd in 512-1023) so one ACT exp covers both
heads.

Schedule: qi-outer attention rounds (round qi = 4 head-pair blocks); each
pair's q/k token-quarter projections are emitted just before its block
(q quarters map 1:1 to rounds), v groups just before the round that needs
them, and ready out-projection groups drain as fillers inside later blocks
so the PE never idles while ACT chews exps. Attention blocks are
software-pipelined three steps deep (ao matmuls for step kt emitted after
the exp of step kt+3; scores rotate through 3 PSUM buffers, ao through 1).
DMA issue is spread across the sync/gpsimd/scalar queues so descriptor
generation parallelizes, critical transfers before bulk. The
out-projection is one K=128 matmul per head pair accumulating in PSUM;
its epilogue is a single DVE bias-add, keeping ACT exclusively on exps.
"""
import numpy as np
from contextlib import ExitStack

import ml_dtypes

B, S, D, H = 4, 2048, 1024, 16
HD = 64            # head dim
HPC = 8            # heads per core
F = HPC * HD       # 512 features per head-group
QT = 512           # q tile (free dim)
NQI = S // QT      # 4
NKT = S // 128     # 16
NDK = D // 128     # 8 contraction tiles for projections
SCALE = HD ** -0.5

_CACHE = {}


def _build():
    import concourse.bacc as bacc
    import concourse.tile as tile
    import concourse.mybir as mybir

    f32 = mybir.dt.float32
    bf16 = mybir.dt.bfloat16
    EXP = mybir.ActivationFunctionType.Exp

    nc = bacc.Bacc("TRN2", target_bir_lowering=False, debug=False)
    xT = nc.dram_tensor("xT", [D, S], bf16, kind="ExternalInput").ap()
    w_sl = nc.dram_tensor("w_sl", [D, 3 * F], bf16, kind="ExternalInput").ap()
    wo_sl = nc.dram_tensor("wo_sl", [F, D], bf16, kind="ExternalInput").ap()
    bias_t = nc.dram_tensor("bias_t", [128, 8], f32, kind="ExternalInput").ap()
    mask2 = nc.dram_tensor("mask2", [128, 256], bf16, kind="ExternalInput").ap()
    out = nc.dram_tensor("out", [D, S], bf16, kind="ExternalOutput").ap()

    with tile.TileContext(nc) as tc:
        with ExitStack() as ctx:
            misc = ctx.enter_context(tc.tile_pool(name="misc", bufs=1))
            mask_sb = misc.tile([128, 256], bf16, name="mask_sb", tag="mask")
            bias_sb = misc.tile([128, 8], f32, name="bias_sb", tag="bias")

            pwo = ctx.enter_context(tc.tile_pool(name="pwo", bufs=1))
            wo_t = [pwo.tile([128, D], bf16, name=f"wo{g}", tag=f"wo{g}")
                    for g in range(4)]

            # ---- x / w tiles. DMA issue is spread across engine queues so
            # descriptor generation parallelizes (the Sync sequencer alone
            # serializes at ~600ns/transfer): sync feeds the fast-start x
            # quarter 0, gpsimd feeds the merged q|k|v weight tiles + wo,
            # scalar feeds the bulk x quarters 1-3 (3KB lines).
            xw = ctx.enter_context(tc.tile_pool(name="xw", bufs=1))
            x_t = [xw.tile([128, S], bf16, name=f"x{kk}", tag=f"x{kk}")
                   for kk in range(NDK)]
            w_t = [xw.tile([128, 3 * F], bf16, name=f"w{kk}", tag=f"w{kk}")
                   for kk in range(NDK)]
            # A transfer's latency is its line count on one ring, so the
            # round-0-critical pieces are split across rows (parallel rings)
            # and issued from four engine queues concurrently.
            # Critical stream first, in parallel across the three DMA-capable
            # queues; bulk (x quarters 1-3, wo) strictly after so it queues
            # behind the critical pieces in the per-ring FIFOs.
            for kk in range(NDK):
                r0 = slice(kk * 128, (kk + 1) * 128)
                nc.sync.dma_start(x_t[kk][:, 0:QT], xT[r0, 0:QT])
                eng = nc.gpsimd if kk < 4 else nc.scalar
                for h in range(2):
                    rw = slice(kk * 128 + h * 64, kk * 128 + (h + 1) * 64)
                    eng.dma_start(w_t[kk][h * 64:(h + 1) * 64, 0:2 * F],
                                  w_sl[rw, 0:2 * F])
            # wv spread over all three queues so it lands right behind the
            # critical stream (~16us), just ahead of the first v matmuls
            for kk in range(NDK):
                r0 = slice(kk * 128, (kk + 1) * 128)
                eng = (nc.scalar if kk < 3 else
                       nc.sync if kk < 6 else nc.gpsimd)
                eng.dma_start(w_t[kk][:, 2 * F:3 * F],
                              w_sl[r0, 2 * F:3 * F])
            nc.gpsimd.dma_start(mask_sb[:], mask2)
            nc.gpsimd.dma_start(bias_sb[:], bias_t)
            for kk in range(NDK):
                r0 = slice(kk * 128, (kk + 1) * 128)
                nc.sync.dma_start(x_t[kk][:, QT:2 * QT], xT[r0, QT:2 * QT])
            for kk in range(NDK):
                r0 = slice(kk * 128, (kk + 1) * 128)
                nc.sync.dma_start(x_t[kk][:, 2 * QT:S], xT[r0, 2 * QT:S])
            # wo rides the end of the sync issue queue (~19us in) so its
            # long transfers never contend with the wv/x critical rings;
            # first use (out-proj fillers) is ~100us in
            for g in range(4):
                nc.sync.dma_start(wo_t[g][:], wo_sl[g * 128:(g + 1) * 128, :])

            pqk = ctx.enter_context(tc.tile_pool(name="pqk", bufs=1))
            pv = ctx.enter_context(tc.tile_pool(name="pv", bufs=1))
            patt = ctx.enter_context(tc.tile_pool(name="patt", bufs=1))
            pP = ctx.enter_context(tc.tile_pool(name="pP", bufs=5))
            pr = ctx.enter_context(tc.tile_pool(name="pr", bufs=2))
            prr = ctx.enter_context(tc.tile_pool(name="prr", bufs=2))
            pstg = ctx.enter_context(tc.tile_pool(name="pstg", bufs=3))

            q_sb = [pqk.tile([128, S], bf16, name=f"q{g}", tag=f"q{g}")
                    for g in range(4)]
            k_sb = [pqk.tile([128, S], bf16, name=f"k{g}", tag=f"k{g}")
                    for g in range(4)]
            v_sb = [pv.tile([128, HPC * (HD + 1)], bf16, name=f"v{t}",
                            tag=f"v{t}") for t in range(NKT)]
            am_t = {}
            for g in range(4):
                for qi in range(NQI):
                    am_t[(g, qi)] = patt.tile(
                        [128, QT], bf16, name=f"am{g}{qi}", tag=f"am{g}{qi}")

            # PSUM: tag "big" = scores + qk/v/out-proj tiles (3 bufs, 6
            # banks); tag "ao" = attention accumulators (1 buf, 2 banks).
            psum = ctx.enter_context(
                tc.tile_pool(name="psum", bufs=2, space="PSUM"))

            def v_group(t2):
                ps = psum.tile([128, 2 * QT], f32, name=f"pv{t2}", tag="big", bufs=3)
                for kk in range(NDK):
                    for j in range(2):
                        tt = 2 * t2 + j
                        nc.tensor.matmul(
                            ps[:, j * F:j * F + F],
                            x_t[kk][:, tt * 128:(tt + 1) * 128],
                            w_t[kk][:, 2 * F:3 * F],
                            start=(kk == 0), stop=(kk == NDK - 1))
                for j in range(2):
                    tt = 2 * t2 + j
                    vv = v_sb[tt].rearrange("p (h c) -> p h c", h=HPC)
                    pp = ps[:, j * F:j * F + F].rearrange(
                        "p (h c) -> p h c", h=HPC)
                    nc.vector.tensor_copy(vv[:, :, 0:HD], pp[:])
                    nc.vector.memset(vv[:, :, HD:HD + 1], 1.0)

            def qk_q(g, part, tg):
                """One token quarter of q (part=0) or k (part=1) for pair g."""
                dest = q_sb if part == 0 else k_sb
                fcol = part * F + g * 128
                ps = psum.tile([128, QT], f32,
                               name=f"pq{part}{g}{tg}", tag="big", bufs=3)
                for kk in range(NDK):
                    nc.tensor.matmul(
                        ps[:], w_t[kk][:, fcol:fcol + 128],
                        x_t[kk][:, tg * QT:(tg + 1) * QT],
                        start=(kk == 0), stop=(kk == NDK - 1))
                nc.vector.tensor_copy(
                    dest[g][:, tg * QT:(tg + 1) * QT], ps[:])

            def outproj(dt, qi):
                """One D-row block (128) for one q tile: am rows 0:128 are
                the head pair's full feature set, so each pair is a single
                K=128 matmul; epilogue = one DVE bias-add from PSUM."""
                dcol = slice(dt * 128, dt * 128 + 128)
                ps = psum.tile([128, QT], f32, name=f"op{dt}{qi}", tag="big", bufs=3)
                for pg in range(4):
                    nc.tensor.matmul(
                        ps[:], wo_t[pg][:, dcol], am_t[(pg, qi)][:],
                        start=(pg == 0), stop=(pg == 3))
                s2 = pstg.tile([128, QT], bf16, name=f"s2{dt}{qi}", tag="s2")
                nc.vector.tensor_scalar_add(s2[:], ps[:],
                                            bias_sb[:, dt:dt + 1])
                nc.sync.dma_start(
                    out[dt * 128:(dt + 1) * 128,
                        qi * QT:(qi + 1) * QT], s2[:])

            fillers = []

            def drain(n):
                for _ in range(n):
                    if fillers:
                        fillers.pop(0)()

            def attn_block(pg, qi):
                """Scores + exp + attn@v + normalize for head pair pg,
                q-range [qi*QT, (qi+1)*QT). Software-pipelined: ao matmuls
                for step kt are emitted after exp of step kt+1; one filler
                unit drains every other step."""
                nkt = 4 * qi + 4
                qs = qi * QT
                he, ho = 2 * pg, 2 * pg + 1
                C = HD + 1
                ao = psum.tile([HD + 1, 2 * QT], f32,
                               name=f"ao{pg}{qi}", tag="ao", bufs=1)
                pend = []

                def emit_ao(pt, kt, n0):
                    st = (kt == 0)
                    sp = (kt == nkt - 1)
                    nc.tensor.matmul(
                        ao[:, n0:QT], v_sb[kt][:, he * C:(he + 1) * C],
                        pt[:, n0:QT], start=st, stop=sp)
                    nc.tensor.matmul(
                        ao[:, QT + n0:2 * QT],
                        v_sb[kt][:, ho * C:(ho + 1) * C],
                        pt[:, QT + n0:2 * QT], start=st, stop=sp)

                for kt in range(nkt):
                    d = kt - 4 * qi
                    n0 = 0 if d < 0 else 128 * d
                    kcol = slice(kt * 128, kt * 128 + 128)
                    sc = psum.tile([128, 2 * QT], f32,
                                   name=f"sc{pg}{qi}{kt}", tag="big", bufs=3)
                    nc.tensor.matmul(
                        sc[:, n0:QT], k_sb[pg][0:64, kcol],
                        q_sb[pg][0:64, qs + n0:qs + QT],
                        start=True, stop=True)
                    nc.tensor.matmul(
                        sc[:, QT + n0:2 * QT], k_sb[pg][64:128, kcol],
                        q_sb[pg][64:128, qs + n0:qs + QT],
                        start=True, stop=True)
                    pt = pP.tile([128, 2 * QT], bf16,
                                 name=f"pt{pg}{qi}{kt}", tag="P")
                    sc3 = sc.rearrange("p (h c) -> p h c", h=2)
                    pt3 = pt.rearrange("p (h c) -> p h c", h=2)
                    nc.scalar.activation(pt3[:, :, n0:QT], sc3[:, :, n0:QT],
                                         EXP, scale=SCALE)
                    if d >= 0:
                        m3 = mask_sb.rearrange("p (h c) -> p h c", h=2)
                        nc.vector.tensor_mul(pt3[:, :, n0:n0 + 128],
                                             pt3[:, :, n0:n0 + 128], m3[:])
                    if len(pend) == 3:
                        emit_ao(*pend.pop(0))
                    if kt % 4 == 1:
                        drain(1)
                    pend.append((pt, kt, n0))
                for p in pend:
                    emit_ao(*p)

                # normalize: 1/rowsum (row HD) via fast recip + gpsimd bcast,
                # split into halves so the two broadcasts pipeline with the
                # copies/recips instead of serializing the whole chain
                am = am_t[(pg, qi)]
                srow = prr.tile([1, 2 * QT], f32, name=f"sr{pg}{qi}", tag="sr")
                rb = pr.tile([HD, 2 * QT], f32, name=f"rb{pg}{qi}", tag="r")
                for h in range(2):
                    cs = slice(h * QT, (h + 1) * QT)
                    nc.vector.tensor_copy(srow[:, cs], ao[HD:HD + 1, cs])
                    nc.vector.reciprocal_approx_fast(srow[:, cs],
                                                     srow[:, cs])
                    nc.gpsimd.partition_broadcast(rb[:, cs], srow[:, cs],
                                                  channels=HD)
                nc.vector.tensor_mul(am[0:64, :], ao[0:HD, 0:QT], rb[:, 0:QT])
                nc.vector.tensor_mul(am[64:128, :], ao[0:HD, QT:2 * QT],
                                     rb[:, QT:2 * QT])

            # ---- schedule ----
            # Round qi: per pair, emit that pair's q/k token-quarter qi just
            # before its block (q quarters map 1:1 to rounds; k quarter qi
            # first needed by the round-qi diagonal). v groups and ready
            # out-proj groups drain as fillers inside the blocks.
            qk_q(0, 0, 0)
            qk_q(0, 1, 0)
            v_group(0)
            v_group(1)
            for qi in range(NQI):
                if qi >= 1:
                    v_group(2 * qi)
                    v_group(2 * qi + 1)
                    # out-proj groups drain one round late (round qi drains
                    # op(qi-2); round 3 also op(2)) so the filler supply
                    # reaches each round's ACT-bound later blocks
                    if qi >= 2:
                        fillers.extend([
                            (lambda dt=dt, q=qi - 2: outproj(dt, q))
                            for dt in range(8)])
                    if qi == 3:
                        fillers.extend([
                            (lambda dt=dt: outproj(dt, 2))
                            for dt in range(8)])
                    qk_q(0, 0, qi)
                    qk_q(0, 1, qi)
                # one-block lookahead: the next pair's q/k units emit before
                # this block so their DVE copies finish under this block's
                # attention instead of stalling the next block's scores
                for g in range(4):
                    if g < 3:
                        qk_q(g + 1, 0, qi)
                        qk_q(g + 1, 1, qi)
                    attn_block(g, qi)
            drain(99)

            # tail: qi=3 out-proj as dt-pairs on 2-bank tiles. The pg0-2
            # accumulation opens before the last block's normalize lands
            # (on both free PSUM tags so neither waits it); pg3 + epilogue
            # close after.
            def op3_open(d0, tag):
                ps = psum.tile([128, 2 * QT], f32, name=f"ot{d0}", tag=tag, bufs=3)
                for j in range(2):
                    dcol = slice((d0 + j) * 128, (d0 + j + 1) * 128)
                    for pg in range(3):
                        nc.tensor.matmul(
                            ps[:, j * QT:(j + 1) * QT], wo_t[pg][:, dcol],
                            am_t[(pg, 3)][:], start=(pg == 0), stop=False)
                return ps

            def op3_close(d0, ps):
                for j in range(2):
                    dcol = slice((d0 + j) * 128, (d0 + j + 1) * 128)
                    nc.tensor.matmul(
                        ps[:, j * QT:(j + 1) * QT], wo_t[3][:, dcol],
                        am_t[(3, 3)][:], start=False, stop=True)
                for j in range(2):
                    dt = d0 + j
                    s2 = pstg.tile([128, QT], bf16, name=f"s3{dt}", tag="s2")
                    nc.vector.tensor_scalar_add(
                        s2[:], ps[:, j * QT:(j + 1) * QT],
                        bias_sb[:, dt:dt + 1])
                    nc.sync.dma_start(
                        out[dt * 128:(dt + 1) * 128, 3 * QT:4 * QT], s2[:])

            ps0 = op3_open(0, "big")
            ps1 = op3_open(2, "big")
            op3_close(0, ps0)
            op3_close(2, ps1)
            ps2 = op3_open(4, "big")
            ps3 = op3_open(6, "big")
            op3_close(4, ps2)
            op3_close(6, ps3)

    nc.compile()
    return nc


def _get_nc():
    if "nc" not in _CACHE:
        _CACHE["nc"] = _build()
    return _CACHE["nc"]


def _prep_inputs(x, w_qkv, w_out, b_out):
    """Build the 8 per-core input maps."""
    x = np.asarray(x, dtype=np.float32)
    w_qkv = np.asarray(w_qkv, dtype=np.float32)
    w_out = np.asarray(w_out, dtype=np.float32)
    b_out = np.asarray(b_out, dtype=np.float32)

    bf = ml_dtypes.bfloat16
    tri = np.triu(np.ones((128, 128), dtype=np.float32))
    mask2 = np.tile(tri, (1, 2)).astype(bf)
    zeros_bias = np.zeros((128, 8), dtype=np.float32)
    bias_t = np.ascontiguousarray(b_out.reshape(8, 128).T)

    in_maps = []
    for c in range(8):
        b, hg = c // 2, c % 2
        cols = hg * F
        w_cat = np.concatenate([
            w_qkv[:, cols:cols + F],
            w_qkv[:, D + cols:D + cols + F],
            w_qkv[:, 2 * D + cols:2 * D + cols + F],
        ], axis=1)
        in_maps.append({
            "xT": np.ascontiguousarray(x[b].T).astype(bf),
            "w_sl": np.ascontiguousarray(w_cat).astype(bf),
            "wo_sl": np.ascontiguousarray(w_out[cols:cols + F, :]).astype(bf),
            "bias_t": bias_t if hg == 0 else zeros_bias,
            "mask2": mask2,
        })
    return in_maps


def _run(inputs, trace=False):
    from concourse.bass_utils import run_bass_kernel_spmd

    nc = _get_nc()
    in_maps = _prep_inputs(**inputs)
    res = run_bass_kernel_spmd(nc, in_maps, core_ids=list(range(8)),
                               trace=trace)
    outs = []
    for b in range(B):
        o = (res.results[2 * b]["out"].astype(np.float32)
             + res.results[2 * b + 1]["out"].astype(np.float32))
        outs.append(o.T)
    full = np.stack(outs).astype(np.float32)
    return full, res


def kernel(x, w_qkv, w_out, b_out):
    full, _ = _run({"x": x, "w_qkv": w_qkv, "w_out": w_out, "b_out": b_out})
    return full
